# revision 1
# baseline (speedup 1.0000x reference)
"""Multi-head attention (T=2048, B=4, E=1024, H=16) on 8 TRN2 NeuronCores.

Sharding: core c = (b, g) with b = c // 2 (batch), g = c % 2 (head-group of 8
heads = feature slice of 512). Each core computes its batch's projections for
its 8 heads, attention, and a partial output projection over its 512 local
features; the host sums the two partials per batch.

Per-core kernel layout (all matmul operands bf16, fp32 PSUM accumulation):
  - host pre-transposes x to [e, t] so projections need no on-chip transpose
  - Q^T, K^T produced as [f, t] (head-pair stacked on partitions)
  - V produced as [j, d] (so it can be the stationary operand of AV)
  - scores computed transposed S^T[j, i] per head, two heads row-tiled
  - softmax: exp(S + mask_bias) on ACT (no max subtraction needed: inputs are
    bounded), denominator via a ones-column appended to V in the AV matmul,
    normalization via DVE reciprocal + GpSimd partition_broadcast + DVE mul
"""

import sys

if "/opt/trn_rl_repo" not in sys.path:
    sys.path.insert(0, "/opt/trn_rl_repo")

import numpy as np
import ml_dtypes

import concourse.bass as bass  # noqa: F401
import concourse.mybir as mybir
import concourse.tile as tile
from concourse import bacc
from concourse import bass_utils

P = 128
TQ = 2048
TK = 2048
E = 1024
EC = E // P          # 8 contraction chunks
NPAIR = 4            # head pairs per core (8 heads)
IB = 512             # i-block (query block)
NI = TQ // IB        # 4
NJ = TK // P         # 16 key chunks
N_CORES = 8

BF = mybir.dt.bfloat16
F32 = mybir.dt.float32
EXP = mybir.ActivationFunctionType.Exp


def build_bass():
    nc = bacc.Bacc("TRN2", target_bir_lowering=False, debug=False,
                   num_devices=N_CORES)
    xq_d = nc.dram_tensor("xq", (E, TQ), BF, kind="ExternalInput").ap()
    xk_d = nc.dram_tensor("xk", (E, TK), BF, kind="ExternalInput").ap()
    xv_d = nc.dram_tensor("xv", (E, TK), BF, kind="ExternalInput").ap()
    wq_d = nc.dram_tensor("wq", (E, 512), BF, kind="ExternalInput").ap()
    wk_d = nc.dram_tensor("wk", (E, 512), BF, kind="ExternalInput").ap()
    wv_d = nc.dram_tensor("wv", (E, 512), BF, kind="ExternalInput").ap()
    wo_d = nc.dram_tensor("wo", (512, E), BF, kind="ExternalInput").ap()
    mb_d = nc.dram_tensor("maskb", (P, NJ), F32, kind="ExternalInput").ap()
    out_d = nc.dram_tensor("out", (TQ, E), F32, kind="ExternalOutput").ap()

    with tile.TileContext(nc) as tc:
        with (
            tc.tile_pool(name="const", bufs=1) as const,
            tc.tile_pool(name="xpool", bufs=4) as xpool,
            tc.tile_pool(name="spool", bufs=4) as spool,
            tc.tile_pool(name="npool", bufs=2) as npool,
            tc.tile_pool(name="ppsum", bufs=2, space="PSUM") as ppsum,
            tc.tile_pool(name="spsum", bufs=2, space="PSUM") as spsum,
            tc.tile_pool(name="apsum", bufs=2, space="PSUM") as apsum,
        ):
            # ---- constants -------------------------------------------------
            wq_sb = const.tile([P, EC, 512], BF)
            nc.sync.dma_start(wq_sb, wq_d.rearrange("(ec p) f -> p ec f", p=P))
            wk_sb = const.tile([P, EC, 512], BF)
            nc.sync.dma_start(wk_sb, wk_d.rearrange("(ec p) f -> p ec f", p=P))
            wv_sb = const.tile([P, EC, 512], BF)
            nc.sync.dma_start(wv_sb, wv_d.rearrange("(ec p) f -> p ec f", p=P))
            wo_sb = const.tile([P, 4, E], BF)
            nc.sync.dma_start(wo_sb, wo_d.rearrange("(ec p) f -> p ec f", p=P))
            mb_sb = const.tile([P, NJ], F32)
            nc.sync.dma_start(mb_sb, mb_d)

            QT = [const.tile([P, TQ], BF, name=f"QT{p}") for p in range(NPAIR)]
            KT = [const.tile([P, TK], BF, name=f"KT{p}") for p in range(NPAIR)]
            Vsb = [const.tile([P, NJ, 2, 66], BF, name=f"Vsb{p}")
                   for p in range(NPAIR)]
            Osb = [const.tile([P, TQ], BF, name=f"Osb{p}") for p in range(NPAIR)]
            for p in range(NPAIR):
                nc.vector.memset(Vsb[p][:, :, :, 64:65], 1.0)

            xq_r = xq_d.rearrange("(ec p) t -> p ec t", p=P)
            xk_r = xk_d.rearrange("(ec p) t -> p ec t", p=P)
            xv_r = xv_d.rearrange("(ec p) t -> p ec t", p=P)

            # ---- projection quanta ----------------------------------------
            def qk_quantum(p, t, x_r, w_sb, dst):
                def emit():
                    xt = xpool.tile([P, EC, IB], BF, tag="x", name="xt")
                    nc.sync.dma_start(xt, x_r[:, :, t * IB:(t + 1) * IB])
                    ps = ppsum.tile([P, 512], F32, tag="pp", name="psqk")
                    for ec in range(EC):
                        nc.tensor.matmul(ps, lhsT=w_sb[:, ec, p * P:(p + 1) * P],
                                         rhs=xt[:, ec, :],
                                         start=(ec == 0), stop=(ec == EC - 1))
                    nc.vector.tensor_copy(dst[:, t * IB:(t + 1) * IB], ps)
                return emit

            def v_quantum(p, jc):
                def emit():
                    xt = xpool.tile([P, EC, P], BF, tag="xv", name="xvt")
                    nc.sync.dma_start(xt, xv_r[:, :, jc * P:(jc + 1) * P])
                    ps = ppsum.tile([P, 512], F32, tag="pp", name="psv")
                    psv = ps[:, 0:P]
                    for ec in range(EC):
                        nc.tensor.matmul(psv, lhsT=xt[:, ec, :],
                                         rhs=wv_sb[:, ec, p * P:(p + 1) * P],
                                         start=(ec == 0), stop=(ec == EC - 1))
                    nc.vector.tensor_copy(
                        Vsb[p][:, jc, :, 0:64],
                        psv.rearrange("p (h d) -> p h d", d=64))
                return emit

            def proj_quanta(p):
                qs = []
                for t in range(NI):
                    qs.append(qk_quantum(p, t, xq_r, wq_sb, QT[p]))
                for t in range(NI):
                    qs.append(qk_quantum(p, t, xk_r, wk_sb, KT[p]))
                for jc in range(NJ):
                    qs.append(v_quantum(p, jc))
                return qs

            # ---- attention for one head pair ------------------------------
            def emit_attention(p, pending):
                for ib in range(NI):
                    avA = apsum.tile([P, 512], F32, tag="av", name="avA")
                    avB = apsum.tile([P, 512], F32, tag="av", name="avB")
                    isl = slice(ib * IB, (ib + 1) * IB)
                    for jc in range(NJ):
                        s = spsum.tile([P, 1024], F32, tag="s", name="s")
                        jsl = slice(jc * P, (jc + 1) * P)
                        nc.tensor.matmul(s[:, 0:512],
                                         lhsT=KT[p][0:64, jsl],
                                         rhs=QT[p][0:64, isl],
                                         start=True, stop=True)
                        nc.tensor.matmul(s[:, 512:1024],
                                         lhsT=KT[p][64:128, jsl],
                                         rhs=QT[p][64:128, isl],
                                         start=True, stop=True)
                        e_sb = spool.tile([P, 1024], BF, tag="exp", name="esb")
                        nc.scalar.activation(e_sb, s, EXP,
                                             bias=mb_sb[:, jc:jc + 1])
                        nc.tensor.matmul(avA[0:65, :],
                                         lhsT=Vsb[p][:, jc, 0, 0:65],
                                         rhs=e_sb[:, 0:512],
                                         start=(jc == 0), stop=(jc == NJ - 1))
                        nc.tensor.matmul(avB[0:65, :],
                                         lhsT=Vsb[p][:, jc, 1, 0:65],
                                         rhs=e_sb[:, 512:1024],
                                         start=(jc == 0), stop=(jc == NJ - 1))
                        if pending and jc % 2 == 1:
                            pending.pop(0)()
                    for h, av in ((0, avA), (1, avB)):
                        rc = npool.tile([1, 512], F32, tag="rc", name="rc")
                        nc.vector.reciprocal(rc, av[64:65, :])
                        rep = npool.tile([64, 512], F32, tag="rep", name="rep")
                        nc.gpsimd.partition_broadcast(rep, rc[0:1, :])
                        nc.vector.tensor_mul(
                            Osb[p][h * 64:(h + 1) * 64, isl],
                            av[0:64, :], rep)

            # ---- main flow -------------------------------------------------
            for fn in proj_quanta(0):
                fn()
            for p in range(NPAIR):
                pending = proj_quanta(p + 1) if p + 1 < NPAIR else []
                emit_attention(p, pending)
                for fn in pending:
                    fn()

            # ---- output projection ----------------------------------------
            for t in range(TQ // P):
                tsl = slice(t * P, (t + 1) * P)
                for fo in range(2):
                    ps = ppsum.tile([P, 512], F32, tag="pp", name="pso")
                    for ec in range(4):
                        nc.tensor.matmul(ps, lhsT=Osb[ec][:, tsl],
                                         rhs=wo_sb[:, ec,
                                                   fo * 512:(fo + 1) * 512],
                                         start=(ec == 0), stop=(ec == 3))
                    st = spool.tile([P, 512], F32, tag="ostage", name="ost")
                    nc.vector.tensor_copy(st, ps)
                    nc.sync.dma_start(out_d[tsl, fo * 512:(fo + 1) * 512], st)

    nc.compile()
    return nc


def make_in_maps(q, k, v, key_padding_mask, Wq, Wk, Wv, Wo):
    bf16 = ml_dtypes.bfloat16
    q = np.asarray(q, dtype=np.float32)
    k = np.asarray(k, dtype=np.float32)
    v = np.asarray(v, dtype=np.float32)
    mask = np.asarray(key_padding_mask)
    Wq = np.asarray(Wq, dtype=np.float32)
    Wk = np.asarray(Wk, dtype=np.float32)
    Wv = np.asarray(Wv, dtype=np.float32)
    Wo = np.asarray(Wo, dtype=np.float32)

    xqT, xkT, xvT, mbias = {}, {}, {}, {}
    for b in range(4):
        xqT[b] = np.ascontiguousarray(q[:, b, :].T).astype(bf16)
        xkT[b] = np.ascontiguousarray(k[:, b, :].T).astype(bf16)
        xvT[b] = np.ascontiguousarray(v[:, b, :].T).astype(bf16)
        bias = np.where(mask[b], np.float32(-1e9), np.float32(0.0))
        mbias[b] = np.ascontiguousarray(
            bias.astype(np.float32).reshape(NJ, P).T)
    wqT, wkT, wvT, woT = {}, {}, {}, {}
    for g in range(2):
        fs = slice(g * 512, (g + 1) * 512)
        wqT[g] = np.ascontiguousarray(Wq[fs, :].T / 8.0).astype(bf16)
        wkT[g] = np.ascontiguousarray(Wk[fs, :].T).astype(bf16)
        wvT[g] = np.ascontiguousarray(Wv[fs, :].T).astype(bf16)
        woT[g] = np.ascontiguousarray(Wo[:, fs].T).astype(bf16)

    in_maps = []
    for c in range(N_CORES):
        b, g = divmod(c, 2)
        in_maps.append({
            "xq": xqT[b], "xk": xkT[b], "xv": xvT[b],
            "wq": wqT[g], "wk": wkT[g], "wv": wvT[g], "wo": woT[g],
            "maskb": mbias[b],
        })
    return in_maps


_NC_CACHE = {}


def _get_nc():
    if "nc" not in _NC_CACHE:
        _NC_CACHE["nc"] = build_bass()
    return _NC_CACHE["nc"]


def run(in_maps, trace=False, **kwargs):
    nc = _get_nc()
    return bass_utils.run_bass_kernel_spmd(
        nc, in_maps, core_ids=list(range(N_CORES)), trace=trace, **kwargs)


def kernel(q, k, v, key_padding_mask, Wq, Wk, Wv, Wo):
    in_maps = make_in_maps(q, k, v, key_padding_mask, Wq, Wk, Wv, Wo)
    res = run(in_maps, trace=False)
    out = np.empty((TQ, 4, E), dtype=np.float32)
    for b in range(4):
        out[:, b, :] = res.results[2 * b]["out"] + res.results[2 * b + 1]["out"]
    return out


if __name__ == "__main__":
    nc = build_bass()
    print("build+compile OK")


# revision 6
# speedup vs baseline: 1.2680x; 1.2680x over previous
"""Multi-head attention (T=2048, B=4, E=1024, H=16) on 8 TRN2 NeuronCores.

Sharding: core c = (b, g) with b = c // 2 (batch), g = c % 2 (head-group of 8
heads = feature slice of 512). Each core computes its batch's projections for
its 8 heads, attention, and a partial output projection over its 512 local
features; the host sums the two partials per batch.

Per-core kernel layout (all matmul operands bf16, fp32 PSUM accumulation):
  - host pre-transposes x to [e, t] so projections need no on-chip transpose
  - Q^T, K^T produced as [f, t] (head-pair stacked on partitions)
  - V produced as [j, d] (so it can be the stationary operand of AV)
  - scores computed transposed S^T[j, i] per head, two heads row-tiled
  - softmax: exp(S + mask_bias) on ACT (no max subtraction needed: inputs are
    bounded), denominator via a ones-column appended to V in the AV matmul,
    normalization via DVE reciprocal + GpSimd partition_broadcast + DVE mul
"""

import sys

if "/opt/trn_rl_repo" not in sys.path:
    sys.path.insert(0, "/opt/trn_rl_repo")

import numpy as np
import ml_dtypes

import concourse.bass as bass  # noqa: F401
import concourse.mybir as mybir
import concourse.tile as tile
from concourse import bacc
from concourse import bass_utils

P = 128
TQ = 2048
TK = 2048
E = 1024
EC = E // P          # 8 contraction chunks
NPAIR = 4            # head pairs per core (8 heads)
IB = 512             # i-block (query block)
NI = TQ // IB        # 4
NJ = TK // P         # 16 key chunks
N_CORES = 8

BF = mybir.dt.bfloat16
F32 = mybir.dt.float32
EXP = mybir.ActivationFunctionType.Exp


def build_bass():
    nc = bacc.Bacc("TRN2", target_bir_lowering=False, debug=False,
                   num_devices=N_CORES)
    xq_d = nc.dram_tensor("xq", (E, TQ), BF, kind="ExternalInput").ap()
    xk_d = nc.dram_tensor("xk", (E, TK), BF, kind="ExternalInput").ap()
    xv_d = nc.dram_tensor("xv", (E, TK), BF, kind="ExternalInput").ap()
    wq_d = nc.dram_tensor("wq", (E, 512), BF, kind="ExternalInput").ap()
    wk_d = nc.dram_tensor("wk", (E, 512), BF, kind="ExternalInput").ap()
    wv_d = nc.dram_tensor("wv", (E, 512), BF, kind="ExternalInput").ap()
    wo_d = nc.dram_tensor("wo", (512, E), BF, kind="ExternalInput").ap()
    mb_d = nc.dram_tensor("maskb", (P, NJ), F32, kind="ExternalInput").ap()
    out_d = nc.dram_tensor("out", (TQ, E), F32, kind="ExternalOutput").ap()

    with tile.TileContext(nc) as tc:
        with (
            tc.tile_pool(name="const", bufs=1) as const,
            tc.tile_pool(name="xpool", bufs=4) as xpool,
            tc.tile_pool(name="spool", bufs=4) as spool,
            tc.tile_pool(name="npool", bufs=2) as npool,
            tc.tile_pool(name="ppsum", bufs=2, space="PSUM") as ppsum,
            tc.tile_pool(name="spsum", bufs=2, space="PSUM") as spsum,
            tc.tile_pool(name="apsum", bufs=2, space="PSUM") as apsum,
        ):
            # ---- constants -------------------------------------------------
            wq_sb = const.tile([P, EC, 512], BF)
            nc.sync.dma_start(wq_sb, wq_d.rearrange("(ec p) f -> p ec f", p=P))
            wk_sb = const.tile([P, EC, 512], BF)
            nc.sync.dma_start(wk_sb, wk_d.rearrange("(ec p) f -> p ec f", p=P))
            wv_sb = const.tile([P, EC, 512], BF)
            nc.sync.dma_start(wv_sb, wv_d.rearrange("(ec p) f -> p ec f", p=P))
            wo_sb = const.tile([P, 4, E], BF)
            nc.sync.dma_start(wo_sb, wo_d.rearrange("(ec p) f -> p ec f", p=P))
            mb_sb = const.tile([P, NJ], F32)
            nc.sync.dma_start(mb_sb, mb_d)

            QT = [const.tile([P, TQ], BF, name=f"QT{p}") for p in range(NPAIR)]
            KT = [const.tile([P, TK], BF, name=f"KT{p}") for p in range(NPAIR)]
            Vsb = const.tile([P, NJ, 8, 66], BF)
            Osb = [const.tile([P, TQ], BF, name=f"Osb{p}") for p in range(NPAIR)]
            nc.vector.memset(Vsb[:, :, :, 64:65], 1.0)

            xq_r = xq_d.rearrange("(ec p) t -> p ec t", p=P)
            xk_r = xk_d.rearrange("(ec p) t -> p ec t", p=P)
            xv_r = xv_d.rearrange("(ec p) t -> p ec t", p=P)

            # ---- projection quanta ----------------------------------------
            def qk_quantum(p, t, x_r, w_sb, dst):
                def emit():
                    xt = xpool.tile([P, EC, IB], BF, tag="x", name="xt")
                    nc.sync.dma_start(xt, x_r[:, :, t * IB:(t + 1) * IB])
                    ps = ppsum.tile([P, 512], F32, tag="pp", name="psqk")
                    for ec in range(EC):
                        nc.tensor.matmul(ps, lhsT=w_sb[:, ec, p * P:(p + 1) * P],
                                         rhs=xt[:, ec, :],
                                         start=(ec == 0), stop=(ec == EC - 1))
                    nc.vector.tensor_copy(dst[:, t * IB:(t + 1) * IB], ps)
                return emit

            def v_quantum(jc):
                # full-width V projection for all 8 heads at key chunk jc
                def emit():
                    xt = xpool.tile([P, EC, P], BF, tag="xv", name="xvt")
                    nc.sync.dma_start(xt, xv_r[:, :, jc * P:(jc + 1) * P])
                    ps = ppsum.tile([P, 512], F32, tag="pp", name="psv")
                    for ec in range(EC):
                        nc.tensor.matmul(ps, lhsT=xt[:, ec, :],
                                         rhs=wv_sb[:, ec, :],
                                         start=(ec == 0), stop=(ec == EC - 1))
                    nc.vector.tensor_copy(
                        Vsb[:, jc, :, 0:64],
                        ps.rearrange("p (h d) -> p h d", d=64))
                return emit

            def proj_quanta(p):
                qs = []
                for t in range(NI):
                    qs.append(qk_quantum(p, t, xq_r, wq_sb, QT[p]))
                for t in range(NI):
                    qs.append(qk_quantum(p, t, xk_r, wk_sb, KT[p]))
                return qs

            # ---- attention for one head pair ------------------------------
            def emit_attention(p, pending, per_j=2):
                for ib in range(NI):
                    avA = apsum.tile([P, 512], F32, tag="av", name="avA")
                    avB = apsum.tile([P, 512], F32, tag="av", name="avB")
                    isl = slice(ib * IB, (ib + 1) * IB)
                    for jc in range(NJ):
                        if pending and jc % per_j == per_j - 1:
                            pending.pop(0)()
                        s = spsum.tile([P, 1024], F32, tag="s", name="s")
                        jsl = slice(jc * P, (jc + 1) * P)
                        nc.tensor.matmul(s[:, 0:512],
                                         lhsT=KT[p][0:64, jsl],
                                         rhs=QT[p][0:64, isl],
                                         start=True, stop=True)
                        nc.tensor.matmul(s[:, 512:1024],
                                         lhsT=KT[p][64:128, jsl],
                                         rhs=QT[p][64:128, isl],
                                         start=True, stop=True)
                        e_sb = spool.tile([P, 1024], BF, tag="exp", name="esb")
                        nc.scalar.activation(e_sb, s, EXP,
                                             bias=mb_sb[:, jc:jc + 1])
                        nc.tensor.matmul(avA[0:65, :],
                                         lhsT=Vsb[:, jc, 2 * p, 0:65],
                                         rhs=e_sb[:, 0:512],
                                         start=(jc == 0), stop=(jc == NJ - 1))
                        nc.tensor.matmul(avB[0:65, :],
                                         lhsT=Vsb[:, jc, 2 * p + 1, 0:65],
                                         rhs=e_sb[:, 512:1024],
                                         start=(jc == 0), stop=(jc == NJ - 1))
                    for h, av in ((0, avA), (1, avB)):
                        # copy out of PSUM promptly so the next block's AV
                        # accumulators can allocate; normalize off-path.
                        # (denominator moves to partition 0 first — the
                        # custom-DVE approx reciprocal miscompiles on
                        # non-zero base partitions)
                        raw = npool.tile([64, 512], F32, tag="raw", name="raw")
                        nc.vector.tensor_copy(raw, av[0:64, :])
                        dn = npool.tile([1, 512], F32, tag="dn", name="dn")
                        nc.vector.tensor_copy(dn, av[64:65, :])
                        rc = npool.tile([1, 512], F32, tag="rc", name="rc")
                        nc.vector.reciprocal_approx_fast(rc, dn)
                        rep = npool.tile([64, 512], F32, tag="rep", name="rep")
                        nc.gpsimd.partition_broadcast(rep, rc[0:1, :])
                        nc.vector.tensor_mul(
                            Osb[p][h * 64:(h + 1) * 64, isl],
                            raw, rep)

            # ---- main flow -------------------------------------------------
            # attn(p) interleaves projection work for pair p+1; attn(0) also
            # consumes the V-projection quanta just-in-time (V[jc] is emitted
            # right before its first consumer, AV at jc of the first i-block)
            for fn in proj_quanta(0):
                fn()
            for p in range(NPAIR):
                pending = []
                if p == 0:
                    pending += [v_quantum(jc) for jc in range(NJ)]
                if p + 1 < NPAIR:
                    pending += proj_quanta(p + 1)
                emit_attention(p, pending, per_j=1 if p == 0 else 2)
                for fn in pending:
                    fn()

            # ---- output projection ----------------------------------------
            for t in range(TQ // P):
                tsl = slice(t * P, (t + 1) * P)
                for fo in range(2):
                    ps = ppsum.tile([P, 512], F32, tag="pp", name="pso")
                    for ec in range(4):
                        nc.tensor.matmul(ps, lhsT=Osb[ec][:, tsl],
                                         rhs=wo_sb[:, ec,
                                                   fo * 512:(fo + 1) * 512],
                                         start=(ec == 0), stop=(ec == 3))
                    st = spool.tile([P, 512], F32, tag="ostage", name="ost")
                    nc.vector.tensor_copy(st, ps)
                    nc.sync.dma_start(out_d[tsl, fo * 512:(fo + 1) * 512], st)

    nc.compile()
    return nc


def make_in_maps(q, k, v, key_padding_mask, Wq, Wk, Wv, Wo):
    bf16 = ml_dtypes.bfloat16
    q = np.asarray(q, dtype=np.float32)
    k = np.asarray(k, dtype=np.float32)
    v = np.asarray(v, dtype=np.float32)
    mask = np.asarray(key_padding_mask)
    Wq = np.asarray(Wq, dtype=np.float32)
    Wk = np.asarray(Wk, dtype=np.float32)
    Wv = np.asarray(Wv, dtype=np.float32)
    Wo = np.asarray(Wo, dtype=np.float32)

    xqT, xkT, xvT, mbias = {}, {}, {}, {}
    for b in range(4):
        xqT[b] = np.ascontiguousarray(q[:, b, :].T).astype(bf16)
        xkT[b] = np.ascontiguousarray(k[:, b, :].T).astype(bf16)
        xvT[b] = np.ascontiguousarray(v[:, b, :].T).astype(bf16)
        bias = np.where(mask[b], np.float32(-1e9), np.float32(0.0))
        mbias[b] = np.ascontiguousarray(
            bias.astype(np.float32).reshape(NJ, P).T)
    wqT, wkT, wvT, woT = {}, {}, {}, {}
    for g in range(2):
        fs = slice(g * 512, (g + 1) * 512)
        wqT[g] = np.ascontiguousarray(Wq[fs, :].T / 8.0).astype(bf16)
        wkT[g] = np.ascontiguousarray(Wk[fs, :].T).astype(bf16)
        wvT[g] = np.ascontiguousarray(Wv[fs, :].T).astype(bf16)
        woT[g] = np.ascontiguousarray(Wo[:, fs].T).astype(bf16)

    in_maps = []
    for c in range(N_CORES):
        b, g = divmod(c, 2)
        in_maps.append({
            "xq": xqT[b], "xk": xkT[b], "xv": xvT[b],
            "wq": wqT[g], "wk": wkT[g], "wv": wvT[g], "wo": woT[g],
            "maskb": mbias[b],
        })
    return in_maps


_NC_CACHE = {}


def _get_nc():
    if "nc" not in _NC_CACHE:
        _NC_CACHE["nc"] = build_bass()
    return _NC_CACHE["nc"]


def run(in_maps, trace=False, **kwargs):
    nc = _get_nc()
    return bass_utils.run_bass_kernel_spmd(
        nc, in_maps, core_ids=list(range(N_CORES)), trace=trace, **kwargs)


def kernel(q, k, v, key_padding_mask, Wq, Wk, Wv, Wo):
    in_maps = make_in_maps(q, k, v, key_padding_mask, Wq, Wk, Wv, Wo)
    res = run(in_maps, trace=False)
    out = np.empty((TQ, 4, E), dtype=np.float32)
    for b in range(4):
        out[:, b, :] = res.results[2 * b]["out"] + res.results[2 * b + 1]["out"]
    return out


if __name__ == "__main__":
    nc = build_bass()
    print("build+compile OK")
